# revision 19
# baseline (speedup 1.0000x reference)
"""Trainium2 Bass kernel for nn_PointerAttention (head-mean pointer logits).

Reference computation (B=4, T=2048, S=4096, D=512, H=8, HD=64):
    q = query @ q_w.T + q_b
    k = keys  @ k_w.T + k_b
    logits[b,t,s] = sum_d q[b,t,d] * k[b,s,d] / (H * sqrt(HD))   # = /64
    logits = where(mask[b,s], -inf, logits)

Algebraic refactor (all folding done on host in float64):
    Q = X Wq^T + 1 bq^T ;  K = Y Wk^T + 1 bk^T
    Q K^T = X (Wq^T Wk) Y^T + 1 (Y Wk^T bq)^T + (X Wq^T bk + bq.bk) 1^T
    Let  M = Wq^T Wk / 64          [D, D]
         v = Wk^T bq / 64          [D]     (per-partition bias of stage 1)
         w = (X (Wq^T bk) + bq.bk)/64  [T] per batch (per-partition bias, stage 2)
    Then out = (X M + 1 v^T) Y^T + w 1^T
       stage 1 (device): Q2T[e,t] = sum_c M[c,e] xT[c,t] + v[e]
       stage 2 (device): out[t,s] = sum_e Q2T[e,t] yT[e,s] + w[t]
    where xT = query[b].T and yT = keys[b].T are RAW inputs — only one
    projection-sized matmul remains and the K-side projection disappears.

Sharding: 8 cores = 4 batches x 2 key-column halves. Each core computes
out[b, :, half] = [2048, 2048] (16 MiB). No collectives.

Matmuls run in float32r (1 cycle/row on TRN2 PE vs 4 for float32).
"""

from contextlib import ExitStack

import numpy as np

import concourse.bass as bass  # noqa: F401  (bass types used via tile/bacc)
import concourse.tile as tile
from concourse import bacc, mybir
from concourse.bass_utils import run_bass_kernel_spmd

# Problem dims (hardcoded; harness contract)
B, T, S, D = 4, 2048, 4096, 512
SCALE = 64.0  # N_HEADS * sqrt(HEAD_DIM) = 8 * 8
N_CORES = 8
SHALF = S // 2  # keys columns per core
P = 128  # SBUF partitions
FD = 512  # matmul moving free dim == one fp32 PSUM bank
KC = D // P  # contraction chunks (4)
NT_TILES = T // P  # output row tiles per core (16)
NS_CHUNKS = SHALF // FD  # output col chunks per core (4)
NT_CHUNKS = T // FD  # stage-1 moving chunks (4)

_NC_CACHE: dict = {}


def _alloc(ctx: ExitStack, tc):
    f32 = mybir.dt.float32
    f32r = mybir.dt.float32r
    persist = ctx.enter_context(tc.tile_pool(name="persist", bufs=1))
    psum = ctx.enter_context(tc.tile_pool(name="psum", bufs=8, space="PSUM"))
    ostage = ctx.enter_context(tc.tile_pool(name="ostage", bufs=3))
    tiles = {
        "psum": psum,
        "ostage": ostage,
        "m": [persist.tile([P, D], f32r, tag=f"m{c}", name=f"m{c}") for c in range(KC)],
        "x": [persist.tile([P, T], f32r, tag=f"x{c}", name=f"x{c}") for c in range(KC)],
        "y": [
            persist.tile([P, SHALF], f32r, tag=f"y{e}", name=f"y{e}")
            for e in range(KC)
        ],
        "q2": [
            persist.tile([P, T], f32r, tag=f"q2{e}", name=f"q2{e}") for e in range(KC)
        ],
        "v": persist.tile([P, KC], f32, tag="v", name="vt"),
        "w": persist.tile([P, NT_TILES], f32, tag="w", name="wt"),
        "warm": persist.tile([P, 256], f32r, tag="warm", name="warm"),
        "warm_f32": persist.tile([P, 256], f32, tag="warm_f32", name="warm_f32"),
    }
    return tiles


def _emit_body(tiles, tc, xT, yT, m, v, w, out):
    nc = tc.nc
    ident = mybir.ActivationFunctionType.Identity
    psum, ostage = tiles["psum"], tiles["ostage"]
    m_t, x_t, y_t, q2_t = tiles["m"], tiles["x"], tiles["y"], tiles["q2"]
    v_t, w_t = tiles["v"], tiles["w"]

    nc.sync.dma_start(v_t[:], v[:])
    nc.sync.dma_start(w_t[:], w[:])

    # PE warmup: ~16 junk matmuls during the initial DMA wait so the HAM
    # clock-gate reaches 8/8 before the first real matmul.
    warm = tiles["warm"]
    warm_f32 = tiles["warm_f32"]
    wps = tiles["psum"].tile([P, 256], mybir.dt.float32, tag="wps", name="wps", bufs=1)
    nc.vector.memset(warm_f32[:], 0.0)
    nc.vector.tensor_copy(warm[:], warm_f32[:])
    for i in range(16):
        nc.tensor.matmul(
            wps[:], warm[:, 0:P], warm[:], start=(i == 0), stop=(i == 15)
        )

    # Input loads straight into float32r tiles (DRAM tensors are declared
    # f32r, so the DMACopy producer satisfies walrus' fp32r check).
    # Order = consumption order: M in stage-1 e-order, x in stage-1 n-order
    # (keeps stage 1 fed), then y in stage-2 s-order (s=0 cols of every
    # e-tile arrive first). Inputs ride the SP HWDGE ring; outputs ride the
    # ACT ring (separate FIFO, no head-of-line blocking between the two).
    for c in range(KC):
        nc.sync.dma_start(m_t[c][:], m[c * P:(c + 1) * P, :])
    for n in range(NT_CHUNKS):
        for c in range(KC):
            nc.sync.dma_start(
                x_t[c][:, n * FD:(n + 1) * FD],
                xT[c * P:(c + 1) * P, n * FD:(n + 1) * FD],
            )
    for s in range(NS_CHUNKS):
        for e in range(KC):
            nc.sync.dma_start(
                y_t[e][:, s * FD:(s + 1) * FD],
                yT[e * P:(e + 1) * P, s * FD:(s + 1) * FD],
            )

    # Stage 1 chunk emitter: Q2T[e, t] = sum_c M[c,e] xT[c,t] + v[e]
    def stage1(n):
        for e in range(KC):
            ps = psum.tile([P, FD], mybir.dt.float32, tag="ps", name="ps", bufs=7)
            for c in range(KC):
                nc.tensor.matmul(
                    ps[:],
                    m_t[c][:, e * P:(e + 1) * P],
                    x_t[c][:, n * FD:(n + 1) * FD],
                    start=(c == 0),
                    stop=(c == KC - 1),
                )
            # eviction rounds to f32r for the stage-2 matmul; alternate engines
            if e % 2 == 0:
                nc.scalar.activation(
                    q2_t[e][:, n * FD:(n + 1) * FD], ps[:], ident, bias=v_t[:, e:e + 1]
                )
            else:
                nc.vector.tensor_scalar_add(
                    q2_t[e][:, n * FD:(n + 1) * FD], ps[:], v_t[:, e:e + 1]
                )

    # Stage 2 tile emitter: out[t, s] = sum_e Q2T[e,t] yT[e,s] + w[t]
    def stage2(tt):
        ot = ostage.tile([P, SHALF], mybir.dt.float32, tag="ot", name="ot")
        for s in range(NS_CHUNKS):
            ps = psum.tile([P, FD], mybir.dt.float32, tag="ps", name="ps", bufs=7)
            for e in range(KC):
                nc.tensor.matmul(
                    ps[:],
                    q2_t[e][:, tt * P:(tt + 1) * P],
                    y_t[e][:, s * FD:(s + 1) * FD],
                    start=(e == 0),
                    stop=(e == KC - 1),
                )
            if (tt + s) % 2 == 0:
                nc.scalar.activation(
                    ot[:, s * FD:(s + 1) * FD], ps[:], ident, bias=w_t[:, tt:tt + 1]
                )
            else:
                nc.vector.tensor_scalar_add(
                    ot[:, s * FD:(s + 1) * FD], ps[:], w_t[:, tt:tt + 1]
                )
            if tt == NT_TILES - 1:
                # last row-tile: ship each s-chunk as soon as it's evicted so
                # the kernel tail is one 256 KB drain, not a 1 MiB one
                nc.scalar.dma_start(
                    out[tt * P:(tt + 1) * P, s * FD:(s + 1) * FD],
                    ot[:, s * FD:(s + 1) * FD],
                )
        if tt < NT_TILES - 1:
            nc.scalar.dma_start(out[tt * P:(tt + 1) * P, :], ot[:])

    # PE program order: all of stage 1 (its span covers the y DMA window),
    # then stage 2.
    for n in range(NT_CHUNKS):
        stage1(n)
    for tt in range(NT_TILES):
        stage2(tt)


def _build(reps: int = 1, loop_reps: int = 1):
    """Build + compile the per-core Bass program. reps>1 statically unrolls
    the whole body; loop_reps>1 wraps it in a runtime For_i loop (both are
    used only for timing measurements)."""
    key = (reps, loop_reps)
    if key in _NC_CACHE:
        return _NC_CACHE[key]
    nc = bacc.Bacc(trn_type="TRN2", target_bir_lowering=False, debug=False)
    f32 = mybir.dt.float32
    f32r = mybir.dt.float32r
    xT = nc.dram_tensor("xT", [D, T], f32r, kind="ExternalInput").ap()
    yT = nc.dram_tensor("yT", [D, SHALF], f32r, kind="ExternalInput").ap()
    m = nc.dram_tensor("m", [D, D], f32r, kind="ExternalInput").ap()
    v = nc.dram_tensor("v", [P, KC], f32, kind="ExternalInput").ap()
    w = nc.dram_tensor("w", [P, NT_TILES], f32, kind="ExternalInput").ap()
    out = nc.dram_tensor("out", [T, SHALF], f32, kind="ExternalOutput").ap()
    with tile.TileContext(nc) as tc:
        with ExitStack() as ctx:
            tiles = _alloc(ctx, tc)
            if loop_reps > 1:
                hint = (
                    mybir.EngineType.PE,
                    mybir.EngineType.Activation,
                    mybir.EngineType.DVE,
                    mybir.EngineType.SP,
                )
                with tc.For_i(0, loop_reps, 1, hint_engines=hint):
                    for _ in range(reps):
                        _emit_body(tiles, tc, xT, yT, m, v, w, out)
            else:
                for _ in range(reps):
                    _emit_body(tiles, tc, xT, yT, m, v, w, out)
    nc.compile()
    _NC_CACHE[key] = nc
    return nc


def _host_prep(query, keys, q_w, q_b, k_w, k_b):
    """Fold weights/biases on host (float64), build per-core input maps."""
    q_w64 = np.asarray(q_w, np.float64)
    k_w64 = np.asarray(k_w, np.float64)
    q_b64 = np.asarray(q_b, np.float64)
    k_b64 = np.asarray(k_b, np.float64)

    m_in = np.ascontiguousarray(((q_w64.T @ k_w64) / SCALE).astype(np.float32))
    v64 = (k_w64.T @ q_b64) / SCALE  # [D]
    v_in = np.ascontiguousarray(v64.astype(np.float32).reshape(KC, P).T)
    g = q_w64.T @ k_b64  # [D]
    cc = float(q_b64 @ k_b64)
    # w[b, t] = (query[b] @ g + bq.bk) / 64
    w_all = ((np.asarray(query, np.float64) @ g + cc) / SCALE).astype(np.float32)

    in_maps = []
    for i in range(N_CORES):
        b, h = divmod(i, N_CORES // B)
        in_maps.append(
            {
                "xT": np.ascontiguousarray(query[b].T),
                "yT": np.ascontiguousarray(keys[b, h * SHALF:(h + 1) * SHALF, :].T),
                "m": m_in,
                "v": v_in,
                "w": np.ascontiguousarray(w_all[b].reshape(NT_TILES, P).T),
            }
        )
    return in_maps


def _gather(results, mask):
    out = np.empty((B, T, S), np.float32)
    for i in range(N_CORES):
        b, h = divmod(i, N_CORES // B)
        out[b, :, h * SHALF:(h + 1) * SHALF] = results[i]["out"]
    if mask is not None and mask.any():
        out = np.where(mask[:, None, :], np.float32(-np.inf), out)
    return out


def kernel(query, keys, key_padding_mask, q_w, q_b, k_w, k_b):
    query = np.asarray(query, np.float32)
    keys = np.asarray(keys, np.float32)
    mask = np.asarray(key_padding_mask, bool)
    assert query.shape == (B, T, D) and keys.shape == (B, S, D)

    in_maps = _host_prep(query, keys, q_w, q_b, k_w, k_b)
    nc = _build(reps=1)
    res = run_bass_kernel_spmd(nc, in_maps, core_ids=list(range(N_CORES)))
    return _gather(res.results, mask)


# revision 20
# speedup vs baseline: 1.0172x; 1.0172x over previous
"""Trainium2 Bass kernel for nn_PointerAttention (head-mean pointer logits).

Reference computation (B=4, T=2048, S=4096, D=512, H=8, HD=64):
    q = query @ q_w.T + q_b
    k = keys  @ k_w.T + k_b
    logits[b,t,s] = sum_d q[b,t,d] * k[b,s,d] / (H * sqrt(HD))   # = /64
    logits = where(mask[b,s], -inf, logits)

Algebraic refactor (all folding done on host in float64):
    Q = X Wq^T + 1 bq^T ;  K = Y Wk^T + 1 bk^T
    Q K^T = X (Wq^T Wk) Y^T + 1 (Y Wk^T bq)^T + (X Wq^T bk + bq.bk) 1^T
    Let  M = Wq^T Wk / 64          [D, D]
         v = Wk^T bq / 64          [D]     (per-partition bias of stage 1)
         w = (X (Wq^T bk) + bq.bk)/64  [T] per batch (per-partition bias, stage 2)
    Then out = (X M + 1 v^T) Y^T + w 1^T
       stage 1 (device): Q2T[e,t] = sum_c M[c,e] xT[c,t] + v[e]
       stage 2 (device): out[t,s] = sum_e Q2T[e,t] yT[e,s] + w[t]
    where xT = query[b].T and yT = keys[b].T are RAW inputs — only one
    projection-sized matmul remains and the K-side projection disappears.

Sharding: 8 cores = 4 batches x 2 key-column halves. Each core computes
out[b, :, half] = [2048, 2048] (16 MiB). No collectives.

Matmuls run in float32r (1 cycle/row on TRN2 PE vs 4 for float32).
"""

import os
from contextlib import ExitStack

import numpy as np

import concourse.bass as bass  # noqa: F401  (bass types used via tile/bacc)
import concourse.tile as tile
from concourse import bacc, mybir
from concourse.bass_utils import run_bass_kernel_spmd

# Problem dims (hardcoded; harness contract)
B, T, S, D = 4, 2048, 4096, 512
SCALE = 64.0  # N_HEADS * sqrt(HEAD_DIM) = 8 * 8
N_CORES = 8
SHALF = S // 2  # keys columns per core
P = 128  # SBUF partitions
FD = 512  # matmul moving free dim == one fp32 PSUM bank
KC = D // P  # contraction chunks (4)
NT_TILES = T // P  # output row tiles per core (16)
NS_CHUNKS = SHALF // FD  # output col chunks per core (4)
NT_CHUNKS = T // FD  # stage-1 moving chunks (4)

_NC_CACHE: dict = {}

# experiment toggles (timing A/B only; defaults are the shipped config)
K_WARMUP = os.environ.get("K_WARMUP", "1") == "1"
K_OUT_RING = os.environ.get("K_OUT_RING", "act")


def _alloc(ctx: ExitStack, tc):
    f32 = mybir.dt.float32
    f32r = mybir.dt.float32r
    persist = ctx.enter_context(tc.tile_pool(name="persist", bufs=1))
    psum = ctx.enter_context(tc.tile_pool(name="psum", bufs=8, space="PSUM"))
    ostage = ctx.enter_context(tc.tile_pool(name="ostage", bufs=3))
    tiles = {
        "psum": psum,
        "ostage": ostage,
        "m": [persist.tile([P, D], f32r, tag=f"m{c}", name=f"m{c}") for c in range(KC)],
        "x": [persist.tile([P, T], f32r, tag=f"x{c}", name=f"x{c}") for c in range(KC)],
        "y": [
            persist.tile([P, SHALF], f32r, tag=f"y{e}", name=f"y{e}")
            for e in range(KC)
        ],
        "q2": [
            persist.tile([P, T], f32r, tag=f"q2{e}", name=f"q2{e}") for e in range(KC)
        ],
        "v": persist.tile([P, KC], f32, tag="v", name="vt"),
        "w": persist.tile([P, NT_TILES], f32, tag="w", name="wt"),
        "warm": persist.tile([P, 256], f32r, tag="warm", name="warm"),
        "warm_f32": persist.tile([P, 256], f32, tag="warm_f32", name="warm_f32"),
    }
    return tiles


def _emit_body(tiles, tc, xT, yT, m, v, w, out):
    nc = tc.nc
    ident = mybir.ActivationFunctionType.Identity
    psum, ostage = tiles["psum"], tiles["ostage"]
    m_t, x_t, y_t, q2_t = tiles["m"], tiles["x"], tiles["y"], tiles["q2"]
    v_t, w_t = tiles["v"], tiles["w"]

    nc.sync.dma_start(v_t[:], v[:])
    nc.sync.dma_start(w_t[:], w[:])

    # PE warmup: ~16 junk matmuls during the initial DMA wait so the HAM
    # clock-gate reaches 8/8 before the first real matmul.
    warm = tiles["warm"]
    warm_f32 = tiles["warm_f32"]
    wps = tiles["psum"].tile([P, 256], mybir.dt.float32, tag="wps", name="wps", bufs=1)
    if K_WARMUP:
        nc.vector.memset(warm_f32[:], 0.0)
        nc.vector.tensor_copy(warm[:], warm_f32[:])
        for i in range(16):
            nc.tensor.matmul(
                wps[:], warm[:, 0:P], warm[:], start=(i == 0), stop=(i == 15)
            )

    # Input loads straight into float32r tiles (DRAM tensors are declared
    # f32r, so the DMACopy producer satisfies walrus' fp32r check).
    # Order = consumption order: M in stage-1 e-order, x in stage-1 n-order
    # (keeps stage 1 fed), then y in stage-2 s-order (s=0 cols of every
    # e-tile arrive first). Inputs ride the SP HWDGE ring; outputs ride the
    # ACT ring (separate FIFO, no head-of-line blocking between the two).
    for c in range(KC):
        nc.sync.dma_start(m_t[c][:], m[c * P:(c + 1) * P, :])
    for n in range(NT_CHUNKS):
        for c in range(KC):
            nc.sync.dma_start(
                x_t[c][:, n * FD:(n + 1) * FD],
                xT[c * P:(c + 1) * P, n * FD:(n + 1) * FD],
            )
    for s in range(NS_CHUNKS):
        for e in range(KC):
            nc.sync.dma_start(
                y_t[e][:, s * FD:(s + 1) * FD],
                yT[e * P:(e + 1) * P, s * FD:(s + 1) * FD],
            )

    # Stage 1 chunk emitter: Q2T[e, t] = sum_c M[c,e] xT[c,t] + v[e]
    def stage1(n):
        for e in range(KC):
            ps = psum.tile([P, FD], mybir.dt.float32, tag="ps", name="ps", bufs=7)
            for c in range(KC):
                nc.tensor.matmul(
                    ps[:],
                    m_t[c][:, e * P:(e + 1) * P],
                    x_t[c][:, n * FD:(n + 1) * FD],
                    start=(c == 0),
                    stop=(c == KC - 1),
                )
            # eviction rounds to f32r for the stage-2 matmul; alternate engines
            if e % 2 == 0:
                nc.scalar.activation(
                    q2_t[e][:, n * FD:(n + 1) * FD], ps[:], ident, bias=v_t[:, e:e + 1]
                )
            else:
                nc.vector.tensor_scalar_add(
                    q2_t[e][:, n * FD:(n + 1) * FD], ps[:], v_t[:, e:e + 1]
                )

    out_eng = nc.scalar if K_OUT_RING == "act" else nc.sync

    # Stage 2 tile emitter: out[t, s] = sum_e Q2T[e,t] yT[e,s] + w[t]
    def stage2(tt):
        ot = ostage.tile([P, SHALF], mybir.dt.float32, tag="ot", name="ot")
        for s in range(NS_CHUNKS):
            ps = psum.tile([P, FD], mybir.dt.float32, tag="ps", name="ps", bufs=7)
            for e in range(KC):
                nc.tensor.matmul(
                    ps[:],
                    q2_t[e][:, tt * P:(tt + 1) * P],
                    y_t[e][:, s * FD:(s + 1) * FD],
                    start=(e == 0),
                    stop=(e == KC - 1),
                )
            if (tt + s) % 2 == 0:
                nc.scalar.activation(
                    ot[:, s * FD:(s + 1) * FD], ps[:], ident, bias=w_t[:, tt:tt + 1]
                )
            else:
                nc.vector.tensor_scalar_add(
                    ot[:, s * FD:(s + 1) * FD], ps[:], w_t[:, tt:tt + 1]
                )
            if tt == NT_TILES - 1:
                # last row-tile: ship each s-chunk as soon as it's evicted so
                # the kernel tail is one 256 KB drain, not a 1 MiB one
                out_eng.dma_start(
                    out[tt * P:(tt + 1) * P, s * FD:(s + 1) * FD],
                    ot[:, s * FD:(s + 1) * FD],
                )
        if tt < NT_TILES - 1:
            out_eng.dma_start(out[tt * P:(tt + 1) * P, :], ot[:])

    # PE program order: all of stage 1 (its span covers the y DMA window),
    # then stage 2.
    for n in range(NT_CHUNKS):
        stage1(n)
    for tt in range(NT_TILES):
        stage2(tt)


def _build(reps: int = 1, loop_reps: int = 1):
    """Build + compile the per-core Bass program. reps>1 statically unrolls
    the whole body; loop_reps>1 wraps it in a runtime For_i loop (both are
    used only for timing measurements)."""
    key = (reps, loop_reps)
    if key in _NC_CACHE:
        return _NC_CACHE[key]
    nc = bacc.Bacc(trn_type="TRN2", target_bir_lowering=False, debug=False)
    f32 = mybir.dt.float32
    f32r = mybir.dt.float32r
    xT = nc.dram_tensor("xT", [D, T], f32r, kind="ExternalInput").ap()
    yT = nc.dram_tensor("yT", [D, SHALF], f32r, kind="ExternalInput").ap()
    m = nc.dram_tensor("m", [D, D], f32r, kind="ExternalInput").ap()
    v = nc.dram_tensor("v", [P, KC], f32, kind="ExternalInput").ap()
    w = nc.dram_tensor("w", [P, NT_TILES], f32, kind="ExternalInput").ap()
    out = nc.dram_tensor("out", [T, SHALF], f32, kind="ExternalOutput").ap()
    with tile.TileContext(nc) as tc:
        with ExitStack() as ctx:
            tiles = _alloc(ctx, tc)
            if loop_reps > 1:
                hint = (
                    mybir.EngineType.PE,
                    mybir.EngineType.Activation,
                    mybir.EngineType.DVE,
                    mybir.EngineType.SP,
                )
                with tc.For_i(0, loop_reps, 1, hint_engines=hint):
                    for _ in range(reps):
                        _emit_body(tiles, tc, xT, yT, m, v, w, out)
            else:
                for _ in range(reps):
                    _emit_body(tiles, tc, xT, yT, m, v, w, out)
    nc.compile()
    _NC_CACHE[key] = nc
    return nc


def _host_prep(query, keys, q_w, q_b, k_w, k_b):
    """Fold weights/biases on host (float64), build per-core input maps."""
    q_w64 = np.asarray(q_w, np.float64)
    k_w64 = np.asarray(k_w, np.float64)
    q_b64 = np.asarray(q_b, np.float64)
    k_b64 = np.asarray(k_b, np.float64)

    m_in = np.ascontiguousarray(((q_w64.T @ k_w64) / SCALE).astype(np.float32))
    v64 = (k_w64.T @ q_b64) / SCALE  # [D]
    v_in = np.ascontiguousarray(v64.astype(np.float32).reshape(KC, P).T)
    g = q_w64.T @ k_b64  # [D]
    cc = float(q_b64 @ k_b64)
    # w[b, t] = (query[b] @ g + bq.bk) / 64
    w_all = ((np.asarray(query, np.float64) @ g + cc) / SCALE).astype(np.float32)

    in_maps = []
    for i in range(N_CORES):
        b, h = divmod(i, N_CORES // B)
        in_maps.append(
            {
                "xT": np.ascontiguousarray(query[b].T),
                "yT": np.ascontiguousarray(keys[b, h * SHALF:(h + 1) * SHALF, :].T),
                "m": m_in,
                "v": v_in,
                "w": np.ascontiguousarray(w_all[b].reshape(NT_TILES, P).T),
            }
        )
    return in_maps


def _gather(results, mask):
    out = np.empty((B, T, S), np.float32)
    for i in range(N_CORES):
        b, h = divmod(i, N_CORES // B)
        out[b, :, h * SHALF:(h + 1) * SHALF] = results[i]["out"]
    if mask is not None and mask.any():
        out = np.where(mask[:, None, :], np.float32(-np.inf), out)
    return out


def kernel(query, keys, key_padding_mask, q_w, q_b, k_w, k_b):
    query = np.asarray(query, np.float32)
    keys = np.asarray(keys, np.float32)
    mask = np.asarray(key_padding_mask, bool)
    assert query.shape == (B, T, D) and keys.shape == (B, S, D)

    in_maps = _host_prep(query, keys, q_w, q_b, k_w, k_b)
    nc = _build(reps=1)
    res = run_bass_kernel_spmd(nc, in_maps, core_ids=list(range(N_CORES)))
    return _gather(res.results, mask)


# revision 22
# speedup vs baseline: 1.0875x; 1.0691x over previous
"""Trainium2 Bass kernel for nn_PointerAttention (head-mean pointer logits).

Reference computation (B=4, T=2048, S=4096, D=512, H=8, HD=64):
    q = query @ q_w.T + q_b
    k = keys  @ k_w.T + k_b
    logits[b,t,s] = sum_d q[b,t,d] * k[b,s,d] / (H * sqrt(HD))   # = /64
    logits = where(mask[b,s], -inf, logits)

Algebraic refactor (all folding done on host in float64):
    Q = X Wq^T + 1 bq^T ;  K = Y Wk^T + 1 bk^T
    Q K^T = X (Wq^T Wk) Y^T + 1 (Y Wk^T bq)^T + (X Wq^T bk + bq.bk) 1^T
    Let  M = Wq^T Wk / 64          [D, D]
         v = Wk^T bq / 64          [D]     (per-partition bias of stage 1)
         w = (X (Wq^T bk) + bq.bk)/64  [T] per batch (per-partition bias, stage 2)
    Then out = (X M + 1 v^T) Y^T + w 1^T
       stage 1 (device): Q2T[e,t] = sum_c M[c,e] xT[c,t] + v[e]
       stage 2 (device): out[t,s] = sum_e Q2T[e,t] yT[e,s] + w[t]
    where xT = query[b].T and yT = keys[b].T are RAW inputs — only one
    projection-sized matmul remains and the K-side projection disappears.

Sharding: 8 cores = 4 batches x 2 key-column halves. Each core computes
out[b, :, half] = [2048, 2048] (16 MiB). No collectives.

Matmuls run in float32r (1 cycle/row on TRN2 PE vs 4 for float32).
"""

import os
from contextlib import ExitStack

import numpy as np

import concourse.bass as bass  # noqa: F401  (bass types used via tile/bacc)
import concourse.tile as tile
from concourse import bacc, mybir
from concourse.bass_utils import run_bass_kernel_spmd

# Problem dims (hardcoded; harness contract)
B, T, S, D = 4, 2048, 4096, 512
SCALE = 64.0  # N_HEADS * sqrt(HEAD_DIM) = 8 * 8
N_CORES = 8
SHALF = S // 2  # keys columns per core
P = 128  # SBUF partitions
FD = 512  # matmul moving free dim == one fp32 PSUM bank
KC = D // P  # contraction chunks (4)
NT_TILES = T // P  # output row tiles per core (16)
NS_CHUNKS = SHALF // FD  # output col chunks per core (4)
NT_CHUNKS = T // FD  # stage-1 moving chunks (4)

_NC_CACHE: dict = {}

# experiment toggles (timing A/B only; defaults are the shipped config)
K_WARMUP = os.environ.get("K_WARMUP", "0") == "1"
K_OUT_RING = os.environ.get("K_OUT_RING", "act")
K_NO_OUT = os.environ.get("K_NO_OUT", "0") == "1"   # timing ablation only
K_EVICT = os.environ.get("K_EVICT", "split")        # split | act | dve


def _alloc(ctx: ExitStack, tc):
    f32 = mybir.dt.float32
    f32r = mybir.dt.float32r
    persist = ctx.enter_context(tc.tile_pool(name="persist", bufs=1))
    psum = ctx.enter_context(tc.tile_pool(name="psum", bufs=8, space="PSUM"))
    ostage = ctx.enter_context(tc.tile_pool(name="ostage", bufs=3))
    tiles = {
        "psum": psum,
        "ostage": ostage,
        "m": [persist.tile([P, D], f32r, tag=f"m{c}", name=f"m{c}") for c in range(KC)],
        "x": [persist.tile([P, T], f32r, tag=f"x{c}", name=f"x{c}") for c in range(KC)],
        "y": [
            persist.tile([P, SHALF], f32r, tag=f"y{e}", name=f"y{e}")
            for e in range(KC)
        ],
        "q2": [
            persist.tile([P, T], f32r, tag=f"q2{e}", name=f"q2{e}") for e in range(KC)
        ],
        "v": persist.tile([P, KC], f32, tag="v", name="vt"),
        "w": persist.tile([P, NT_TILES], f32, tag="w", name="wt"),
        "warm": persist.tile([P, 256], f32r, tag="warm", name="warm"),
        "warm_f32": persist.tile([P, 256], f32, tag="warm_f32", name="warm_f32"),
    }
    return tiles


def _emit_body(tiles, tc, xT, yT, m, v, w, out):
    nc = tc.nc
    ident = mybir.ActivationFunctionType.Identity
    psum, ostage = tiles["psum"], tiles["ostage"]
    m_t, x_t, y_t, q2_t = tiles["m"], tiles["x"], tiles["y"], tiles["q2"]
    v_t, w_t = tiles["v"], tiles["w"]

    nc.sync.dma_start(v_t[:], v[:])
    nc.sync.dma_start(w_t[:], w[:])

    # PE warmup: ~16 junk matmuls during the initial DMA wait so the HAM
    # clock-gate reaches 8/8 before the first real matmul.
    warm = tiles["warm"]
    warm_f32 = tiles["warm_f32"]
    wps = tiles["psum"].tile([P, 256], mybir.dt.float32, tag="wps", name="wps", bufs=1)
    if K_WARMUP:
        nc.vector.memset(warm_f32[:], 0.0)
        nc.vector.tensor_copy(warm[:], warm_f32[:])
        for i in range(16):
            nc.tensor.matmul(
                wps[:], warm[:, 0:P], warm[:], start=(i == 0), stop=(i == 15)
            )

    # Input loads straight into float32r tiles (DRAM tensors are declared
    # f32r, so the DMACopy producer satisfies walrus' fp32r check).
    # Order = consumption order: M in stage-1 e-order, x in stage-1 n-order
    # (keeps stage 1 fed), then y in stage-2 s-order (s=0 cols of every
    # e-tile arrive first). Inputs ride the SP HWDGE ring; outputs ride the
    # ACT ring (separate FIFO, no head-of-line blocking between the two).
    for c in range(KC):
        nc.sync.dma_start(m_t[c][:], m[c * P:(c + 1) * P, :])
    for n in range(NT_CHUNKS):
        for c in range(KC):
            nc.sync.dma_start(
                x_t[c][:, n * FD:(n + 1) * FD],
                xT[c * P:(c + 1) * P, n * FD:(n + 1) * FD],
            )
    for s in range(NS_CHUNKS):
        for e in range(KC):
            nc.sync.dma_start(
                y_t[e][:, s * FD:(s + 1) * FD],
                yT[e * P:(e + 1) * P, s * FD:(s + 1) * FD],
            )

    # Stage 1 chunk emitter: Q2T[e, t] = sum_c M[c,e] xT[c,t] + v[e]
    def stage1(n):
        for e in range(KC):
            ps = psum.tile([P, FD], mybir.dt.float32, tag="ps", name="ps", bufs=7)
            for c in range(KC):
                nc.tensor.matmul(
                    ps[:],
                    m_t[c][:, e * P:(e + 1) * P],
                    x_t[c][:, n * FD:(n + 1) * FD],
                    start=(c == 0),
                    stop=(c == KC - 1),
                )
            # eviction rounds to f32r for the stage-2 matmul; alternate engines
            if K_EVICT == "act" or (K_EVICT == "split" and e % 2 == 0):
                nc.scalar.activation(
                    q2_t[e][:, n * FD:(n + 1) * FD], ps[:], ident, bias=v_t[:, e:e + 1]
                )
            else:
                nc.vector.tensor_scalar_add(
                    q2_t[e][:, n * FD:(n + 1) * FD], ps[:], v_t[:, e:e + 1]
                )

    out_eng = nc.scalar if K_OUT_RING == "act" else nc.sync

    # Stage 2 tile emitter: out[t, s] = sum_e Q2T[e,t] yT[e,s] + w[t]
    def stage2(tt):
        ot = ostage.tile([P, SHALF], mybir.dt.float32, tag="ot", name="ot")
        for s in range(NS_CHUNKS):
            ps = psum.tile([P, FD], mybir.dt.float32, tag="ps", name="ps", bufs=7)
            for e in range(KC):
                nc.tensor.matmul(
                    ps[:],
                    q2_t[e][:, tt * P:(tt + 1) * P],
                    y_t[e][:, s * FD:(s + 1) * FD],
                    start=(e == 0),
                    stop=(e == KC - 1),
                )
            if K_EVICT == "act" or (K_EVICT == "split" and (tt + s) % 2 == 0):
                nc.scalar.activation(
                    ot[:, s * FD:(s + 1) * FD], ps[:], ident, bias=w_t[:, tt:tt + 1]
                )
            else:
                nc.vector.tensor_scalar_add(
                    ot[:, s * FD:(s + 1) * FD], ps[:], w_t[:, tt:tt + 1]
                )
            if tt == NT_TILES - 1:
                # last row-tile: ship each s-chunk as soon as it's evicted so
                # the kernel tail is one 256 KB drain, not a 1 MiB one
                out_eng.dma_start(
                    out[tt * P:(tt + 1) * P, s * FD:(s + 1) * FD],
                    ot[:, s * FD:(s + 1) * FD],
                )
        if tt < NT_TILES - 1 and not K_NO_OUT:
            out_eng.dma_start(out[tt * P:(tt + 1) * P, :], ot[:])

    # PE program order: all of stage 1 (its span covers the y DMA window),
    # then stage 2.
    for n in range(NT_CHUNKS):
        stage1(n)
    for tt in range(NT_TILES):
        stage2(tt)


def _build(reps: int = 1, loop_reps: int = 1):
    """Build + compile the per-core Bass program. reps>1 statically unrolls
    the whole body; loop_reps>1 wraps it in a runtime For_i loop (both are
    used only for timing measurements)."""
    key = (reps, loop_reps)
    if key in _NC_CACHE:
        return _NC_CACHE[key]
    nc = bacc.Bacc(trn_type="TRN2", target_bir_lowering=False, debug=False)
    f32 = mybir.dt.float32
    f32r = mybir.dt.float32r
    xT = nc.dram_tensor("xT", [D, T], f32r, kind="ExternalInput").ap()
    yT = nc.dram_tensor("yT", [D, SHALF], f32r, kind="ExternalInput").ap()
    m = nc.dram_tensor("m", [D, D], f32r, kind="ExternalInput").ap()
    v = nc.dram_tensor("v", [P, KC], f32, kind="ExternalInput").ap()
    w = nc.dram_tensor("w", [P, NT_TILES], f32, kind="ExternalInput").ap()
    out = nc.dram_tensor("out", [T, SHALF], f32, kind="ExternalOutput").ap()
    with tile.TileContext(nc) as tc:
        with ExitStack() as ctx:
            tiles = _alloc(ctx, tc)
            if loop_reps > 1:
                hint = (
                    mybir.EngineType.PE,
                    mybir.EngineType.Activation,
                    mybir.EngineType.DVE,
                    mybir.EngineType.SP,
                )
                with tc.For_i(0, loop_reps, 1, hint_engines=hint):
                    for _ in range(reps):
                        _emit_body(tiles, tc, xT, yT, m, v, w, out)
            else:
                for _ in range(reps):
                    _emit_body(tiles, tc, xT, yT, m, v, w, out)
    nc.compile()
    _NC_CACHE[key] = nc
    return nc


def _host_prep(query, keys, q_w, q_b, k_w, k_b):
    """Fold weights/biases on host (float64), build per-core input maps."""
    q_w64 = np.asarray(q_w, np.float64)
    k_w64 = np.asarray(k_w, np.float64)
    q_b64 = np.asarray(q_b, np.float64)
    k_b64 = np.asarray(k_b, np.float64)

    m_in = np.ascontiguousarray(((q_w64.T @ k_w64) / SCALE).astype(np.float32))
    v64 = (k_w64.T @ q_b64) / SCALE  # [D]
    v_in = np.ascontiguousarray(v64.astype(np.float32).reshape(KC, P).T)
    g = q_w64.T @ k_b64  # [D]
    cc = float(q_b64 @ k_b64)
    # w[b, t] = (query[b] @ g + bq.bk) / 64
    w_all = ((np.asarray(query, np.float64) @ g + cc) / SCALE).astype(np.float32)

    in_maps = []
    for i in range(N_CORES):
        b, h = divmod(i, N_CORES // B)
        in_maps.append(
            {
                "xT": np.ascontiguousarray(query[b].T),
                "yT": np.ascontiguousarray(keys[b, h * SHALF:(h + 1) * SHALF, :].T),
                "m": m_in,
                "v": v_in,
                "w": np.ascontiguousarray(w_all[b].reshape(NT_TILES, P).T),
            }
        )
    return in_maps


def _gather(results, mask):
    out = np.empty((B, T, S), np.float32)
    for i in range(N_CORES):
        b, h = divmod(i, N_CORES // B)
        out[b, :, h * SHALF:(h + 1) * SHALF] = results[i]["out"]
    if mask is not None and mask.any():
        out = np.where(mask[:, None, :], np.float32(-np.inf), out)
    return out


def kernel(query, keys, key_padding_mask, q_w, q_b, k_w, k_b):
    query = np.asarray(query, np.float32)
    keys = np.asarray(keys, np.float32)
    mask = np.asarray(key_padding_mask, bool)
    assert query.shape == (B, T, D) and keys.shape == (B, S, D)

    in_maps = _host_prep(query, keys, q_w, q_b, k_w, k_b)
    nc = _build(reps=1)
    res = run_bass_kernel_spmd(nc, in_maps, core_ids=list(range(N_CORES)))
    return _gather(res.results, mask)
